# revision 1
# baseline (speedup 1.0000x reference)
"""Trainium2 Bass kernel for 4D convolution (3x3x3x3, pad 1, stride 1).

Problem: x (2, 8, 7, 7, 48, 48) f32, conv (8, 648) f32, bias (8,) f32
         -> out (2, 8, 7, 7, 48, 48) f32.

Strategy
--------
Shard 8 ways as (batch b in {0,1}) x (h-chunk hc in {0..3}, 12 rows each).

Per core, the conv is computed as 27 PSUM-accumulated matmuls per output
u-row (one per (i0, i1, i3) kernel offset over the u, v, w axes), with
the h axis and its 3-tap contraction folded into the matmul via a
block-Toeplitz weight matrix:

  K = 112 partitions = (c in 8) x (s in 14)   s = h-window row of the chunk
  M =  96 partitions = (o in 8) x (t in 12)   t = h output row of the chunk
  lhsT[(c,s), (o,t)] = W[o, i0, i1, s-t, i3, c]  for s-t in {0,1,2}, else 0
  rhs [(c,s), (v,w)] = xpad[b, c, u+i0, v+i1, 12*hc+s, w+i3]   (N = 7*48 = 336)

All data is pre-swizzled on the host so every device DMA is fully
contiguous. Matmuls run in float32r (1 col/cycle for N>=256, ~1e-4 rel
err). Bias is fused into the PSUM->SBUF copy on the scalar engine.

Schedule: x arrives as 7 per-u-row tiles and weights as 3 per-i0-group
tiles so matmuls start as soon as the first chunks land; shifts iterate
i0 in (1, 2, 0) order to match. Output streams out per-u. A few scratch
matmuls issued during the DMA fill keep the PE HAM clock warm.
"""

import sys

if "/opt/trn_rl_repo" not in sys.path:
    sys.path.insert(0, "/opt/trn_rl_repo")

import numpy as np

B, C, OC = 2, 8, 8
U, V, H, W = 7, 7, 48, 48
TH = 12            # h output rows per chunk
S = TH + 2         # h window rows per chunk
KP = C * S         # 112 matmul contraction partitions
MP = OC * TH       # 96 matmul output partitions
NCHUNKS = H // TH  # 4
NCORES = B * NCHUNKS
NCOL = V * W       # 336 moving columns per matmul
XROW = (V + 2) * (W + 2)  # 450 free elements per x partition per u-row
XFREE = U * XROW

# Shift iteration order: i0=1 first so u-row 0's matmuls only need the
# first weight group and x rows 0..1.
SH_ORDER = [
    (i0, i1, i3) for i0 in (1, 2, 0) for i1 in range(3) for i3 in range(3)
]

N_WARMUP_MM = 6

_built = {}


def _build_nc(reps=None):
    """Build the per-core Bass module.

    reps: if set, wrap the whole body in a hardware For-loop executing it
    `reps` times — used only for wall-clock-slope benchmarking.
    """
    import contextlib

    import concourse.bacc as bacc
    import concourse.mybir as mybir
    from concourse.tile import TileContext

    F32R = mybir.dt.float32r
    F32 = mybir.dt.float32

    nc = bacc.Bacc(
        "TRN2", target_bir_lowering=False, debug=False, num_devices=NCORES
    )
    xw_d = nc.dram_tensor("xw", [KP, XFREE], F32R, kind="ExternalInput")
    wt_d = nc.dram_tensor("wt", [KP, 27 * MP], F32R, kind="ExternalInput")
    bias_d = nc.dram_tensor("bias", [MP, 1], F32, kind="ExternalInput")
    out_d = nc.dram_tensor("out", [MP, U * NCOL], F32, kind="ExternalOutput")

    with TileContext(nc) as tc:
        with (
            tc.tile_pool(name="sbuf", bufs=1) as pool,
            tc.tile_pool(name="psum", bufs=4, space="PSUM") as pp,
        ):
            loop = tc.For_i(0, reps, 1) if reps is not None else contextlib.nullcontext()
            with loop:
                # PE warmup: scratch matmuls with no DMA dependency keep the
                # HAM activity monitor busy while inputs stream in.
                scr = pool.tile([128, 512], mybir.dt.bfloat16, tag="scr")
                nc.gpsimd.memset(scr[:], 0.0)
                ps_w = pp.tile([128, 512], F32, tag="ps_warm")
                for _ in range(N_WARMUP_MM):
                    nc.tensor.matmul(
                        ps_w[:], scr[:, :128], scr[:], start=True, stop=True
                    )

                # Input tiles, in the order matmuls will need them. x rides
                # the SP HWDGE ring (nc.sync), weights ride the ACT ring
                # (nc.scalar) so the two streams drain in parallel. The very
                # first shift's weights get their own tiny DMA so matmuls can
                # start as soon as x row 0 lands.
                w_first = pool.tile([KP, MP], F32R, tag="wf", name="w_first")
                w_sb = [
                    pool.tile(
                        [KP, (8 if g == 0 else 9) * MP],
                        F32R,
                        tag=f"w{g}",
                        name=f"w_sb{g}",
                    )
                    for g in range(3)
                ]
                x_sb = [
                    pool.tile([KP, XROW], F32R, tag=f"x{u}", name=f"x_sb{u}")
                    for u in range(U)
                ]
                b_sb = pool.tile([MP, 1], F32, tag="b")
                nc.scalar.dma_start(out=w_first[:], in_=wt_d[:, 0:MP])
                nc.sync.dma_start(out=x_sb[0][:], in_=xw_d[:, 0:XROW])
                nc.sync.dma_start(out=x_sb[1][:], in_=xw_d[:, XROW : 2 * XROW])
                nc.scalar.dma_start(out=w_sb[0][:], in_=wt_d[:, MP : 9 * MP])
                nc.scalar.dma_start(out=w_sb[1][:], in_=wt_d[:, 9 * MP : 18 * MP])
                nc.scalar.dma_start(out=w_sb[2][:], in_=wt_d[:, 18 * MP : 27 * MP])
                nc.scalar.dma_start(out=b_sb[:], in_=bias_d[:])
                for u in range(2, U):
                    nc.sync.dma_start(
                        out=x_sb[u][:], in_=xw_d[:, u * XROW : (u + 1) * XROW]
                    )

                def lhsT_for(pos):
                    if pos == 0:
                        return w_first[:]
                    g, j = divmod(pos, 9)
                    if g == 0:
                        return w_sb[0][:, (j - 1) * MP : j * MP]
                    return w_sb[g][:, j * MP : (j + 1) * MP]

                for u in range(U):
                    ps = pp.tile([MP, NCOL], F32, tag="ps")
                    shifts = [
                        (pos, i0, i1, i3)
                        for pos, (i0, i1, i3) in enumerate(SH_ORDER)
                        # u+i0 indexes the padded u axis [0..9); rows 0 and 8
                        # are all-zero and elided from SBUF: skip the matmuls.
                        if 1 <= u + i0 <= 7
                    ]
                    for idx, (pos, i0, i1, i3) in enumerate(shifts):
                        lhsT = lhsT_for(pos)
                        rhs = (
                            x_sb[u + i0 - 1][:]
                            .rearrange("p (v w) -> p v w", v=V + 2)
                            [:, i1 : i1 + V, i3 : i3 + W]
                        )
                        nc.tensor.matmul(
                            ps[:],
                            lhsT,
                            rhs,
                            start=(idx == 0),
                            stop=(idx == len(shifts) - 1),
                        )
                    o_sb = pool.tile([MP, NCOL], F32, tag=f"o{u}")
                    nc.scalar.activation(
                        out=o_sb[:],
                        in_=ps[:],
                        func=mybir.ActivationFunctionType.Identity,
                        bias=b_sb[:],
                    )
                    nc.sync.dma_start(
                        out=out_d[:, u * NCOL : (u + 1) * NCOL], in_=o_sb[:]
                    )

    nc.compile()
    return nc


def _get_nc():
    if "nc" not in _built:
        _built["nc"] = _build_nc()
    return _built["nc"]


def _build_weight_inputs(conv, bias):
    Wr = conv.reshape(OC, 3, 3, 3, 3, C).astype(np.float32)
    wt = np.zeros((C, S, 27, OC, TH), np.float32)
    t = np.arange(TH)
    for pos, (i0, i1, i3) in enumerate(SH_ORDER):
        for i2 in range(3):
            # wt[c, t+i2, pos, o, t] = Wr[o, i0, i1, i2, i3, c]
            # (advanced-index result has the paired (t) axis first)
            wt[:, t + i2, pos, :, t] = np.broadcast_to(
                Wr[:, i0, i1, i2, i3, :].T, (TH, C, OC)
            )
    wt = np.ascontiguousarray(wt.reshape(KP, 27 * MP))
    bias_in = np.ascontiguousarray(
        np.repeat(bias.astype(np.float32), TH).reshape(MP, 1)
    )
    return wt, bias_in


def _build_x_inputs(x):
    # xh: x padded by 1 along h only -> (B, C, U, V, H+2, W)
    xh = np.zeros((B, C, U, V, H + 2, W), np.float32)
    xh[:, :, :, :, 1 : H + 1, :] = x
    xs = []
    for core in range(NCORES):
        b, hc = divmod(core, NCHUNKS)
        slab = xh[b, :, :, :, hc * TH : hc * TH + S, :]  # (C, U, V, S, W)
        xc = np.zeros((C, S, U, V + 2, W + 2), np.float32)
        xc[:, :, :, 1 : V + 1, 1 : W + 1] = slab.transpose(0, 3, 1, 2, 4)
        xs.append(np.ascontiguousarray(xc.reshape(KP, XFREE)))
    return xs


def kernel(x, conv, bias):
    from concourse.bass_utils import run_bass_kernel_spmd

    nc = _get_nc()
    wt, bias_in = _build_weight_inputs(np.asarray(conv), np.asarray(bias))
    xs = _build_x_inputs(np.asarray(x, dtype=np.float32))
    in_maps = [{"xw": xc, "wt": wt, "bias": bias_in} for xc in xs]
    res = run_bass_kernel_spmd(nc, in_maps, core_ids=list(range(NCORES)))

    out = np.empty((B, OC, U, V, H, W), np.float32)
    for core in range(NCORES):
        b, hc = divmod(core, NCHUNKS)
        r = res.results[core]["out"].reshape(OC, TH, U, V, W)
        out[b, :, :, :, hc * TH : (hc + 1) * TH, :] = r.transpose(0, 2, 3, 1, 4)
    return out

